# revision 68
# baseline (speedup 1.0000x reference)
"""Multi-head attention on 8 Trainium2 NeuronCores.

Sharding: core c = (batch n, head-group g); n = c // 4, g = c % 4.
Each core computes attention for its 4 heads of its batch entry plus the
fc_out partial product for those heads' columns of Wo; the host sums the
4 partials per batch (and adds the bias) to unshard.

Per-core pipeline (bf16 matmuls, f32 PSUM accumulation):
  A) q/k/v projections.  xT arrives as 32 small DMAs (4 L-chunks x 8
     e-tiles) and the first j0/j1 chains consume e-tiles as they land, so
     the first scores+exp reach ScalarE within ~7us.  qT/kT are stored
     head-pair-stacked ([d, L], pair halves on partitions 0-63 / 64-127),
     v in [k, d] layout with a ones column per head (accumulates the
     softmax denominator for free during attn@v).
  B) attention in 8 (pair, 512-q-chunk) units.  Per (unit, k-tile): the
     two heads' scores matmuls (K=64, PE row tiles 0/64 - they overlap in
     the PE array) write the two halves of one [128, 1024] PSUM tile, one
     1024-wide exp on ScalarE (the critical engine: 128 such tiles at
     ~1.12us is the kernel's floor) emits both heads' attn weights, and
     two attn@V matmuls accumulate [d+1, 512] per head (row 64 =
     denominator).  Exp emission runs exactly two units ahead of attn@V
     consumption, in consumption order, so the 36-slot exp pool stays full
     without ring-order deadlocks and ScalarE never drains.  Per-unit
     normalization: reciprocal of the 2 denominator rows, DRAM-bounce
     partition-broadcast, multiply; odd heads DMA-shift to partitions
     64-127.
  C) fc_out partials in 512-column chunks woven into the back half (k >=
     8) of the following units' k-loops - late enough that the previous
     unit's normalize chain (3 serial DMAs) has completed and the fc
     matmul never head-of-line-blocks the PE queue - so only the final
     chunk trails the exp stream.  PSUM evacuations all ride VectorE,
     keeping ScalarE exp-only.
"""

import contextlib as _contextlib
import os
import sys

for _p in ("/opt/trn_rl_repo",):
    if _p not in sys.path and os.path.isdir(_p):
        sys.path.insert(0, _p)

import numpy as np
import ml_dtypes

import concourse.bass as bass
import concourse.mybir as mybir
import concourse.tile as tile
from concourse import bacc
from concourse.bass import ds, ts
from concourse.bass_utils import run_bass_kernel_spmd

BF16 = ml_dtypes.bfloat16
F32 = np.float32

EMBED = 1024
HEADS = 16
HD = 64  # head dim
NB = 2  # batch
L = 2048  # sequence length
NCORES = 8
HPG = 4  # heads per core (group)
NPAIRS = 2  # head pairs per core
ET = 8  # e-contraction tiles of 128
LT = L // 128  # 16 k tiles
NLC = 4  # 512-wide l chunks
QC = 512  # q chunk width
NQC = L // QC  # 4

SCALE = 1.0 / np.sqrt(np.float32(EMBED))  # 1/32

EXPP_BUFS = 46  # exp pool: ~3 units in flight (ring == emission order)

LAST_EXEC_TIME_NS = None
LAST_RESULTS = None

_nc_cache = None


def build_nc():
    """Build + compile the per-core Bass program (same program on all cores)."""
    nc = bacc.Bacc("TRN2")
    f32 = mybir.dt.float32
    bf16 = mybir.dt.bfloat16
    EXP = mybir.ActivationFunctionType.Exp

    xT_d = nc.declare_dram_parameter("xT", [128, ET, L], bf16, isOutput=False)
    wqk_d = nc.declare_dram_parameter("wqk", [128, 4, ET, 128], bf16, isOutput=False)
    wv_d = nc.declare_dram_parameter("wv", [128, ET, HPG * HD], bf16, isOutput=False)
    wo_d = nc.declare_dram_parameter("wo", [NPAIRS, ET, 128, 128], bf16, isOutput=False)
    out_d = nc.declare_dram_parameter("out", [EMBED, L], bf16, isOutput=True)
    recip_dram = nc.dram_tensor("recip_dram", [16, QC], bf16)

    # consumption order of (pair, qc) units; emission leads by 2 units
    units = [(0, 0), (0, 1), (0, 2), (0, 3), (1, 0), (1, 1), (1, 2), (1, 3)]

    with tile.TileContext(nc) as tc:
        with (
            tc.tile_pool(name="expp", bufs=EXPP_BUFS) as expp,
            tc.tile_pool(name="singles", bufs=1) as singles,
            tc.tile_pool(name="drowp", bufs=2) as drowp,
            tc.tile_pool(name="rbp", bufs=4) as rbp,
            tc.tile_pool(name="shiftp", bufs=3) as shiftp,
            tc.tile_pool(name="outp", bufs=4) as outp,
            tc.tile_pool(name="denp", bufs=2) as denp,
            tc.tile_pool(name="nump", bufs=4) as nump,
            tc.tile_pool(name="psAV", bufs=2, space="PSUM") as psAV,
        ):
            # scores pool closes once every exp has been consumed (the
            # close is barrier-free then); its banks feed the tail fc pool
            _psS_stack = _contextlib.ExitStack()
            psS = _psS_stack.enter_context(
                tc.tile_pool(name="psS", bufs=2, space="PSUM")
            )
            # projection PSUM pool lives through phase A only; its 2 banks
            # become the fc accumulator pool in phase B
            _psA_stack = _contextlib.ExitStack()
            psA = _psA_stack.enter_context(
                tc.tile_pool(name="psA", bufs=2, space="PSUM")
            )
            fc_pool = {"cur": None}

            # ---- resident SBUF tensors ----
            xT_sb = singles.tile([128, ET, L], bf16, name="xT_sb")
            wqk_sb = singles.tile([128, 4, ET, 128], bf16, name="wqk_sb")
            wv_sb = singles.tile([128, ET, HPG * HD], bf16, name="wv_sb")
            wo_sb = singles.tile([128, NPAIRS, ET, 128], bf16, name="wo_sb")
            qt_sb = singles.tile([128, NPAIRS, L], bf16, name="qt_sb")
            kt_sb = singles.tile([128, NPAIRS, L], bf16, name="kt_sb")
            v_sb = singles.tile([128, LT, HPG, HD + 1], bf16, name="v_sb")
            outTP_sb = singles.tile([128, NPAIRS, L], bf16, name="outTP_sb")

            # ---- input DMAs, ordered + split so compute starts early ----
            # j0/j1 weights split per (j, et-half) so the first chain
            # matmuls wait on ~128KB transfers, not one 512KB block
            for j in (0, 1):
                nc.sync.dma_start(
                    out=wqk_sb[:, j : j + 1, 0:4, :],
                    in_=wqk_d[:][:, j : j + 1, 0:4, :],
                )
            # lc0 sliced per e-tile: the first j0/j1 chains consume e-tiles
            # as they land
            for et in range(4):
                nc.sync.dma_start(out=xT_sb[:, et, 0:QC], in_=xT_d[:][:, et, 0:QC])
            for j in (0, 1):
                nc.sync.dma_start(
                    out=wqk_sb[:, j : j + 1, 4:8, :],
                    in_=wqk_d[:][:, j : j + 1, 4:8, :],
                )
            for et in range(4, ET):
                nc.sync.dma_start(out=xT_sb[:, et, 0:QC], in_=xT_d[:][:, et, 0:QC])
            for lc in range(1, NLC):
                for eth in range(2):
                    nc.sync.dma_start(
                        out=xT_sb[:, ts(eth, 4), ts(lc, QC)],
                        in_=xT_d[:][:, ts(eth, 4), ts(lc, QC)],
                    )
            nc.sync.dma_start(out=wv_sb, in_=wv_d[:])
            nc.sync.dma_start(out=wqk_sb[:, 2:4, :, :], in_=wqk_d[:][:, 2:4, :, :])
            nc.sync.dma_start(out=wo_sb, in_=wo_d[:].rearrange("r t p c -> p r t c"))

            # ---- build-time exp bookkeeping ----
            # Emissions pop from one global queue in consumption order (==
            # exp-pool ring order), gated on projection availability and
            # pool occupancy.  Consumption happens BEFORE emission in every
            # weave step so the PE never idles at a scores wait while it
            # still has attn@V work queued behind it.
            ex_store = {}  # (unit_idx, k) -> exp tile
            emission_order = [(u, k) for u in range(8) for k in range(LT)]
            st = {"em": 0, "consumed": 0}
            qt_done = {0: set(), 1: set()}
            kt_done = {0: set(), 1: set()}

            def emit_scores_exp(ui, k):
                """Both heads' scores for (unit ui, k-tile k) -> one 1024-wide exp.

                The two K=64 matmuls sit on PE row tiles 0 / 64 and overlap."""
                pair, qc = units[ui]
                sc = psS.tile([128, 1024], f32, tag="sc", name="sc")
                for side in range(2):
                    base = side * HD
                    nc.tensor.matmul(
                        sc[:, ts(side, QC)],
                        kt_sb[base : base + HD, pair, ts(k, 128)],
                        qt_sb[base : base + HD, pair, ts(qc, QC)],
                        start=True,
                        stop=True,
                    )
                ex = expp.tile([128, 1024], bf16, tag="exp", name="ex")
                nc.scalar.activation(ex, sc, EXP, scale=float(SCALE))
                ex_store[(ui, k)] = ex

            def try_emit(n=1):
                for _ in range(n):
                    if st["em"] >= 128:
                        return
                    if st["em"] - st["consumed"] >= EXPP_BUFS:
                        return
                    u, k = emission_order[st["em"]]
                    pair, qc = units[u]
                    if qc not in qt_done[pair] or (k // 4) not in kt_done[pair]:
                        return
                    st["em"] += 1
                    emit_scores_exp(u, k)

            av_tiles = {}

            def attnv_step(ui, k):
                """Consume exp (ui, k): two attn@V matmuls (one per head)."""
                pair, qc = units[ui]
                if ui not in av_tiles:
                    av_tiles[ui] = [
                        psAV.tile([128, QC], f32, tag="av", name=f"av{ui}_{s}")
                        for s in range(2)
                    ]
                st["consumed"] += 1
                ex = ex_store.pop((ui, k))
                for side in range(2):
                    h = pair * 2 + side
                    nc.tensor.matmul(
                        av_tiles[ui][side][0 : HD + 1, :],
                        v_sb[:, k, h, :],
                        ex[:, ts(side, QC)],
                        start=(k == 0),
                        stop=(k == LT - 1),
                    )

            def finish_unit(ui, scalar_help=False):
                """Evacuate + normalize unit ui; free its av PSUM banks.

                scalar_help: ScalarE is exp-free by the last two units, so
                its copies free the av banks without queueing on VectorE."""
                pair, qc = units[ui]
                avs = av_tiles.pop(ui)
                denom = denp.tile([2, QC], f32, tag="den", name="den")
                recip = denp.tile([2, QC], f32, tag="rec", name="rec")
                recipb = denp.tile([2, QC], bf16, tag="recb", name="recb")
                nums = []
                # both sides' denominator rows land on partition 64 at
                # different column offsets, then ride ONE DMA hop
                dr = drowp.tile([HD + 1, 2, QC], f32, tag="dr", name="dr")
                for side in range(2):
                    num = nump.tile([HD, QC], bf16, tag="num", name="num")
                    nums.append(num)
                    if scalar_help:
                        nc.scalar.copy(num, avs[side][0:HD, :])
                        nc.scalar.copy(
                            dr[HD : HD + 1, side, :], avs[side][HD : HD + 1, :]
                        )
                    else:
                        nc.vector.tensor_copy(num, avs[side][0:HD, :])
                        nc.vector.tensor_copy(
                            dr[HD : HD + 1, side, :], avs[side][HD : HD + 1, :]
                        )
                nc.sync.dma_start(out=denom, in_=dr[HD : HD + 1, :, :])
                nc.vector.reciprocal_approx_fast(recip, denom)
                nc.vector.tensor_copy(recipb, recip)
                nc.sync.dma_start(out=recip_dram[2 * ui : 2 * ui + 2, :], in_=recipb)
                for side in range(2):
                    h = pair * 2 + side
                    rb = rbp.tile([HD, QC], bf16, tag="rb", name="rb")
                    nc.sync.dma_start(
                        out=rb,
                        in_=recip_dram[2 * ui + side : 2 * ui + side + 1, :].to_broadcast(
                            [HD, QC]
                        ),
                    )
                    if side == 0:
                        nc.vector.tensor_mul(
                            outTP_sb[0:HD, pair, ts(qc, QC)], nums[side], rb
                        )
                    else:
                        tmp = shiftp.tile([HD, QC], bf16, tag="sh", name="sh")
                        nc.vector.tensor_mul(tmp, nums[side], rb)
                        nc.sync.dma_start(
                            out=outTP_sb[HD:128, pair, ts(qc, QC)], in_=tmp
                        )

            fc_state = {}

            def fc_step(qc, scalar_evac=False):
                """One et-chunk of the fc_out partial for q-columns qc*512.."""
                et = fc_state.get(qc, 0)
                if et >= ET:
                    return False
                fc_state[qc] = et + 1
                fps = fc_pool["cur"].tile([128, QC], f32, tag="fc", name=f"fc{qc}_{et}")
                for pair in range(NPAIRS):
                    nc.tensor.matmul(
                        fps,
                        wo_sb[:, pair, et, :],
                        outTP_sb[:, pair, ts(qc, QC)],
                        start=(pair == 0),
                        stop=(pair == NPAIRS - 1),
                    )
                ob = outp.tile([128, QC], bf16, tag="ob", name="ob")
                # ScalarE is exp-free by the time the late chunks evacuate
                if scalar_evac and et % 2 == 0:
                    nc.scalar.copy(ob, fps)
                else:
                    nc.vector.tensor_copy(ob, fps)
                nc.sync.dma_start(out=out_d[ts(et, 128), ts(qc, QC)], in_=ob)
                return True

            # ================= Phase A: projections =================
            nc.vector.memset(v_sb[:, :, :, HD : HD + 1], 1.0)

            # warm the PE clock during the input-DMA wait: dummy matmuls on
            # a memset tile pull the HAM out of the cold p-state before the
            # first real chains
            warm_sb = singles.tile([128, 384], bf16, name="warm_sb")
            nc.vector.memset(warm_sb, 0.0)
            wps = psA.tile([128, QC], f32, tag="ps", name="warm")
            for i in range(16):
                nc.tensor.matmul(
                    wps[:, 0:256], warm_sb[:, 0:128], warm_sb[:, 128:384],
                    start=(i == 0), stop=(i == 15),
                )

            # lc0: j0/j1 chains interleaved per e-tile so each matmul fires
            # as soon as its xT slice lands
            pst01 = [
                psA.tile([128, QC], f32, tag="ps", name=f"qk{j}_0") for j in (0, 1)
            ]
            for et in range(ET):
                for j in (0, 1):
                    nc.tensor.matmul(
                        pst01[j],
                        wqk_sb[:, j, et, :],
                        xT_sb[:, et, 0:QC],
                        start=(et == 0),
                        stop=(et == ET - 1),
                    )
            nc.vector.tensor_copy(qt_sb[:, 0, 0:QC], pst01[0])
            nc.vector.tensor_copy(kt_sb[:, 0, 0:QC], pst01[1])
            qt_done[0].add(0)
            kt_done[0].add(0)

            def proj_chain(j, lc):
                pst = psA.tile([128, QC], f32, tag="ps", name=f"qk{j}_{lc}")
                for et in range(ET):
                    nc.tensor.matmul(
                        pst,
                        wqk_sb[:, j, et, :],
                        xT_sb[:, et, ts(lc, QC)],
                        start=(et == 0),
                        stop=(et == ET - 1),
                    )
                dst = qt_sb if j % 2 == 0 else kt_sb
                nc.vector.tensor_copy(dst[:, j // 2, ts(lc, QC)], pst)
                (qt_done if j % 2 == 0 else kt_done)[j // 2].add(lc)

            try_emit(4)
            for lc in range(1, NLC):
                proj_chain(0, lc)
                proj_chain(1, lc)
                try_emit(4 if lc < 3 else 32)

            # v projections, weaving unit 0's attn@V
            for lt in range(LT):
                pv = psA.tile([128, QC], f32, tag="ps", name=f"v{lt}")
                pv = pv[:, 0 : HPG * HD]
                for et in range(ET):
                    nc.tensor.matmul(
                        pv,
                        xT_sb[:, et, ts(lt, 128)],
                        wv_sb[:, et, :],
                        start=(et == 0),
                        stop=(et == ET - 1),
                    )
                nc.vector.tensor_copy(
                    v_sb[:, lt, :, 0:HD],
                    pv.rearrange("p (h d) -> p h d", h=HPG),
                )
                if lt >= 2:
                    attnv_step(0, lt - 2)
                try_emit(1)

            # j2/j3 (pair-1 q/k projections) interleaved per l-chunk so
            # pair-1 emissions unlock progressively; weave the tail of unit
            # 0 plus unit 1's attn@V
            weave = [(0, 14), (0, 15)] + [(1, k) for k in range(LT)]
            wi = 0

            def weave_step():
                nonlocal wi
                ui, k = weave[wi]
                wi += 1
                attnv_step(ui, k)
                if (ui, k) == (0, 15):
                    finish_unit(0)
                try_emit(1)

            for ci, (lc, j) in enumerate([(lc, j) for lc in range(NLC) for j in (2, 3)]):
                proj_chain(j, lc)
                take = 3 if ci < 2 else 2
                for _ in range(take):
                    if wi < len(weave):
                        weave_step()
            while wi < len(weave):
                weave_step()
            finish_unit(1)
            try_emit(2)

            # phase A projection banks -> fc accumulator banks
            _psA_stack.close()
            _psC_stack = _contextlib.ExitStack()
            fc_pool["cur"] = _psC_stack.enter_context(
                tc.tile_pool(name="psC", bufs=2, space="PSUM")
            )

            # ================= Phase B: remaining units =================
            # Per k-iter: consume exp (ui, k) with 2 attn@V matmuls FIRST
            # (it can never wait on ScalarE - its exp is ~3 units old),
            # weave an fc et-chunk, then emit the next queued scores+exp
            # (which absorbs any wait for the scores PSUM handoff).
            for ui in range(2, 8):
                for k in range(LT):
                    attnv_step(ui, k)
                    if ui >= 5 and k >= 2:
                        fc_step(ui - 5, scalar_evac=(ui >= 6))
                    try_emit(1)
                finish_unit(ui, scalar_help=(ui >= 6))

            assert st["em"] == 128, f"unemitted scores: {st['em']}"
            for qc in range(NQC - 1):  # qc0-2 leftovers (normally none)
                while fc_step(qc, scalar_evac=True):
                    pass

            # tail: last q-chunk's fc on a 6-deep pool (scores + fc banks,
            # both closable barrier-free now).  Pair-0 partials front-load
            # into the accumulators so the PE crunches them while unit 7's
            # normalize chain (3 serial DMA hops) is still producing the
            # pair-1 outTP columns.
            _psC_stack.close()
            _psS_stack.close()
            _psD_stack = _contextlib.ExitStack()
            psD = _psD_stack.enter_context(
                tc.tile_pool(name="psD", bufs=6, space="PSUM")
            )
            accs = [
                psD.tile([128, QC], f32, tag="fcd", name=f"fcd{et}")
                for et in range(6)
            ]
            for et in range(6):
                nc.tensor.matmul(
                    accs[et], wo_sb[:, 0, et, :], outTP_sb[:, 0, ts(3, QC)],
                    start=True, stop=False,
                )

            def fc_tail(et, fps):
                ob = outp.tile([128, QC], bf16, tag="ob", name="ob")
                if et % 2 == 0:
                    nc.scalar.copy(ob, fps)
                else:
                    nc.vector.tensor_copy(ob, fps)
                nc.sync.dma_start(out=out_d[ts(et, 128), ts(3, QC)], in_=ob)

            for et in range(6):
                nc.tensor.matmul(
                    accs[et], wo_sb[:, 1, et, :], outTP_sb[:, 1, ts(3, QC)],
                    start=False, stop=True,
                )
                fc_tail(et, accs[et])
            for et in (6, 7):
                fps = psD.tile([128, QC], f32, tag="fcd", name=f"fcd{et}")
                for pair in range(NPAIRS):
                    nc.tensor.matmul(
                        fps, wo_sb[:, pair, et, :], outTP_sb[:, pair, ts(3, QC)],
                        start=(pair == 0), stop=(pair == NPAIRS - 1),
                    )
                fc_tail(et, fps)
            _psD_stack.close()

            assert len(ex_store) == 0, f"unconsumed exp tiles: {list(ex_store)}"

    nc.compile()
    return nc


def get_nc():
    global _nc_cache
    if _nc_cache is None:
        _nc_cache = build_nc()
    return _nc_cache


def make_core_inputs(x, Wq, Wk, Wv, Wo, bo):
    """Build the 8 per-core input maps from the full-size inputs."""
    x = np.asarray(x, F32)
    Wq = np.asarray(Wq, F32)
    Wk = np.asarray(Wk, F32)
    Wv = np.asarray(Wv, F32)
    Wo = np.asarray(Wo, F32)

    # xT[p, et, l] = x[n].T[et*128 + p, l]
    xT_b = [
        np.ascontiguousarray(x[n].T).reshape(ET, 128, L).transpose(1, 0, 2).astype(BF16)
        for n in range(NB)
    ]

    in_maps = []
    for c in range(NCORES):
        n, g = divmod(c, HPG)
        heads = [g * HPG + i for i in range(HPG)]

        wqk = np.empty((4, EMBED, 128), F32)
        for j in range(4):
            pair, qk = divmod(j, 2)
            hA = heads[2 * pair]
            hB = heads[2 * pair + 1]
            W = Wq if qk == 0 else Wk
            wqk[j, :, 0:HD] = W[hA * HD : (hA + 1) * HD, :].T
            wqk[j, :, HD:128] = W[hB * HD : (hB + 1) * HD, :].T
        wqk8 = wqk.reshape(4, ET, 128, 128).transpose(2, 0, 1, 3).astype(BF16)

        wv = np.concatenate(
            [Wv[h * HD : (h + 1) * HD, :].T for h in heads], axis=1
        )  # [1024, 256]
        wv8 = wv.reshape(ET, 128, HPG * HD).transpose(1, 0, 2).astype(BF16)

        wo = np.empty((NPAIRS, ET, 128, 128), F32)
        for pair in range(NPAIRS):
            hA = heads[2 * pair]
            hB = heads[2 * pair + 1]
            for et in range(ET):
                blk = Wo[et * 128 : (et + 1) * 128, :]
                wo[pair, et, 0:HD, :] = blk[:, hA * HD : (hA + 1) * HD].T
                wo[pair, et, HD:128, :] = blk[:, hB * HD : (hB + 1) * HD].T

        in_maps.append(
            {
                "xT": xT_b[n],
                "wqk": wqk8,
                "wv": wv8,
                "wo": wo.astype(BF16),
            }
        )
    return in_maps


def combine_outputs(results, bo):
    """Sum the per-core fc_out partials, add bias, transpose to [N, L, E]."""
    out = np.empty((NB, L, EMBED), F32)
    for n in range(NB):
        acc = results[n * HPG]["out"].astype(F32)
        for g in range(1, HPG):
            acc = acc + results[n * HPG + g]["out"].astype(F32)
        out[n] = acc.T + np.asarray(bo, F32)
    return out


def kernel(x, Wq, Wk, Wv, Wo, bo):
    global LAST_EXEC_TIME_NS, LAST_RESULTS
    nc = get_nc()
    in_maps = make_core_inputs(x, Wq, Wk, Wv, Wo, bo)
    trace = bool(os.environ.get("KERNEL_TRACE"))
    kw = {}
    if trace:
        kw["trace"] = True
        kw["trace_cores"] = list(range(NCORES))
    res = run_bass_kernel_spmd(nc, in_maps, list(range(NCORES)), **kw)
    LAST_EXEC_TIME_NS = res.exec_time_ns
    LAST_RESULTS = res
    return combine_outputs(res.results, bo)


# revision 69
# speedup vs baseline: 1.0053x; 1.0053x over previous
"""Multi-head attention on 8 Trainium2 NeuronCores.

Sharding: core c = (batch n, head-group g); n = c // 4, g = c % 4.
Each core computes attention for its 4 heads of its batch entry plus the
fc_out partial product for those heads' columns of Wo; the host sums the
4 partials per batch (and adds the bias) to unshard.

Per-core pipeline (bf16 matmuls, f32 PSUM accumulation):
  A) q/k/v projections.  xT arrives as 32 small DMAs (4 L-chunks x 8
     e-tiles) and the first j0/j1 chains consume e-tiles as they land, so
     the first scores+exp reach ScalarE within ~7us.  qT/kT are stored
     head-pair-stacked ([d, L], pair halves on partitions 0-63 / 64-127),
     v in [k, d] layout with a ones column per head (accumulates the
     softmax denominator for free during attn@v).
  B) attention in 8 (pair, 512-q-chunk) units.  Per (unit, k-tile): the
     two heads' scores matmuls (K=64, PE row tiles 0/64 - they overlap in
     the PE array) write the two halves of one [128, 1024] PSUM tile, one
     1024-wide exp on ScalarE (the critical engine: 128 such tiles at
     ~1.12us is the kernel's floor) emits both heads' attn weights, and
     two attn@V matmuls accumulate [d+1, 512] per head (row 64 =
     denominator).  Exp emission runs exactly two units ahead of attn@V
     consumption, in consumption order, so the 36-slot exp pool stays full
     without ring-order deadlocks and ScalarE never drains.  Per-unit
     normalization: reciprocal of the 2 denominator rows, DRAM-bounce
     partition-broadcast, multiply; odd heads DMA-shift to partitions
     64-127.
  C) fc_out partials in 512-column chunks woven into the back half (k >=
     8) of the following units' k-loops - late enough that the previous
     unit's normalize chain (3 serial DMAs) has completed and the fc
     matmul never head-of-line-blocks the PE queue - so only the final
     chunk trails the exp stream.  PSUM evacuations all ride VectorE,
     keeping ScalarE exp-only.
"""

import contextlib as _contextlib
import os
import sys

for _p in ("/opt/trn_rl_repo",):
    if _p not in sys.path and os.path.isdir(_p):
        sys.path.insert(0, _p)

import numpy as np
import ml_dtypes

import concourse.bass as bass
import concourse.mybir as mybir
import concourse.tile as tile
from concourse import bacc
from concourse.bass import ds, ts
from concourse.bass_utils import run_bass_kernel_spmd

BF16 = ml_dtypes.bfloat16
F32 = np.float32

EMBED = 1024
HEADS = 16
HD = 64  # head dim
NB = 2  # batch
L = 2048  # sequence length
NCORES = 8
HPG = 4  # heads per core (group)
NPAIRS = 2  # head pairs per core
ET = 8  # e-contraction tiles of 128
LT = L // 128  # 16 k tiles
NLC = 4  # 512-wide l chunks
QC = 512  # q chunk width
NQC = L // QC  # 4

SCALE = 1.0 / np.sqrt(np.float32(EMBED))  # 1/32

EXPP_BUFS = 46  # exp pool: ~3 units in flight (ring == emission order)

LAST_EXEC_TIME_NS = None
LAST_RESULTS = None

_nc_cache = None


def build_nc():
    """Build + compile the per-core Bass program (same program on all cores)."""
    nc = bacc.Bacc("TRN2")
    f32 = mybir.dt.float32
    bf16 = mybir.dt.bfloat16
    EXP = mybir.ActivationFunctionType.Exp

    xT_d = nc.declare_dram_parameter("xT", [128, ET, L], bf16, isOutput=False)
    wqk_d = nc.declare_dram_parameter("wqk", [128, 4, ET, 128], bf16, isOutput=False)
    wv_d = nc.declare_dram_parameter("wv", [128, ET, HPG * HD], bf16, isOutput=False)
    wo_d = nc.declare_dram_parameter("wo", [NPAIRS, ET, 128, 128], bf16, isOutput=False)
    out_d = nc.declare_dram_parameter("out", [EMBED, L], bf16, isOutput=True)
    recip_dram = nc.dram_tensor("recip_dram", [16, QC], bf16)

    # consumption order of (pair, qc) units; emission leads by 2 units
    units = [(0, 0), (0, 1), (0, 2), (0, 3), (1, 0), (1, 1), (1, 2), (1, 3)]

    with tile.TileContext(nc) as tc:
        with (
            tc.tile_pool(name="expp", bufs=EXPP_BUFS) as expp,
            tc.tile_pool(name="singles", bufs=1) as singles,
            tc.tile_pool(name="drowp", bufs=3) as drowp,
            tc.tile_pool(name="rbp", bufs=4) as rbp,
            tc.tile_pool(name="shiftp", bufs=3) as shiftp,
            tc.tile_pool(name="outp", bufs=4) as outp,
            tc.tile_pool(name="denp", bufs=2) as denp,
            tc.tile_pool(name="nump", bufs=4) as nump,
            tc.tile_pool(name="psAV", bufs=2, space="PSUM") as psAV,
        ):
            # scores pool closes once every exp has been consumed (the
            # close is barrier-free then); its banks feed the tail fc pool
            _psS_stack = _contextlib.ExitStack()
            psS = _psS_stack.enter_context(
                tc.tile_pool(name="psS", bufs=2, space="PSUM")
            )
            # projection PSUM pool lives through phase A only; its 2 banks
            # become the fc accumulator pool in phase B
            _psA_stack = _contextlib.ExitStack()
            psA = _psA_stack.enter_context(
                tc.tile_pool(name="psA", bufs=2, space="PSUM")
            )
            fc_pool = {"cur": None}

            # ---- resident SBUF tensors ----
            xT_sb = singles.tile([128, ET, L], bf16, name="xT_sb")
            wqk_sb = singles.tile([128, 4, ET, 128], bf16, name="wqk_sb")
            wv_sb = singles.tile([128, ET, HPG * HD], bf16, name="wv_sb")
            wo_sb = singles.tile([128, NPAIRS, ET, 128], bf16, name="wo_sb")
            qt_sb = singles.tile([128, NPAIRS, L], bf16, name="qt_sb")
            kt_sb = singles.tile([128, NPAIRS, L], bf16, name="kt_sb")
            v_sb = singles.tile([128, LT, HPG, HD + 1], bf16, name="v_sb")
            outTP_sb = singles.tile([128, NPAIRS, L], bf16, name="outTP_sb")

            # ---- input DMAs, ordered + split so compute starts early ----
            nc.sync.dma_start(out=wqk_sb[:, 0:2, :, :], in_=wqk_d[:][:, 0:2, :, :])
            # lc0 sliced per e-tile: the first j0/j1 chains consume e-tiles
            # as they land
            for et in range(ET):
                nc.sync.dma_start(out=xT_sb[:, et, 0:QC], in_=xT_d[:][:, et, 0:QC])
            for lc in range(1, NLC):
                for eth in range(2):
                    nc.sync.dma_start(
                        out=xT_sb[:, ts(eth, 4), ts(lc, QC)],
                        in_=xT_d[:][:, ts(eth, 4), ts(lc, QC)],
                    )
            nc.sync.dma_start(out=wv_sb, in_=wv_d[:])
            nc.sync.dma_start(out=wqk_sb[:, 2:4, :, :], in_=wqk_d[:][:, 2:4, :, :])
            nc.sync.dma_start(out=wo_sb, in_=wo_d[:].rearrange("r t p c -> p r t c"))

            # ---- build-time exp bookkeeping ----
            # Emissions pop from one global queue in consumption order (==
            # exp-pool ring order), gated on projection availability and
            # pool occupancy.  Consumption happens BEFORE emission in every
            # weave step so the PE never idles at a scores wait while it
            # still has attn@V work queued behind it.
            ex_store = {}  # (unit_idx, k) -> exp tile
            emission_order = [(u, k) for u in range(8) for k in range(LT)]
            st = {"em": 0, "consumed": 0}
            qt_done = {0: set(), 1: set()}
            kt_done = {0: set(), 1: set()}

            def emit_scores_exp(ui, k):
                """Both heads' scores for (unit ui, k-tile k) -> one 1024-wide exp.

                The two K=64 matmuls sit on PE row tiles 0 / 64 and overlap."""
                pair, qc = units[ui]
                sc = psS.tile([128, 1024], f32, tag="sc", name="sc")
                for side in range(2):
                    base = side * HD
                    nc.tensor.matmul(
                        sc[:, ts(side, QC)],
                        kt_sb[base : base + HD, pair, ts(k, 128)],
                        qt_sb[base : base + HD, pair, ts(qc, QC)],
                        start=True,
                        stop=True,
                    )
                ex = expp.tile([128, 1024], bf16, tag="exp", name="ex")
                nc.scalar.activation(ex, sc, EXP, scale=float(SCALE))
                ex_store[(ui, k)] = ex

            def try_emit(n=1):
                for _ in range(n):
                    if st["em"] >= 128:
                        return
                    if st["em"] - st["consumed"] >= EXPP_BUFS:
                        return
                    u, k = emission_order[st["em"]]
                    pair, qc = units[u]
                    if qc not in qt_done[pair] or (k // 4) not in kt_done[pair]:
                        return
                    st["em"] += 1
                    emit_scores_exp(u, k)

            av_tiles = {}

            def attnv_step(ui, k):
                """Consume exp (ui, k): two attn@V matmuls (one per head)."""
                pair, qc = units[ui]
                if ui not in av_tiles:
                    av_tiles[ui] = [
                        psAV.tile([128, QC], f32, tag="av", name=f"av{ui}_{s}")
                        for s in range(2)
                    ]
                st["consumed"] += 1
                ex = ex_store.pop((ui, k))
                for side in range(2):
                    h = pair * 2 + side
                    nc.tensor.matmul(
                        av_tiles[ui][side][0 : HD + 1, :],
                        v_sb[:, k, h, :],
                        ex[:, ts(side, QC)],
                        start=(k == 0),
                        stop=(k == LT - 1),
                    )

            def finish_unit(ui, scalar_help=False):
                """Evacuate + normalize unit ui; free its av PSUM banks.

                scalar_help: ScalarE is exp-free by the last two units, so
                its copies free the av banks without queueing on VectorE."""
                pair, qc = units[ui]
                avs = av_tiles.pop(ui)
                denom = denp.tile([2, QC], f32, tag="den", name="den")
                recip = denp.tile([2, QC], f32, tag="rec", name="rec")
                recipb = denp.tile([2, QC], bf16, tag="recb", name="recb")
                nums = []
                for side in range(2):
                    num = nump.tile([HD, QC], bf16, tag="num", name="num")
                    nums.append(num)
                    if scalar_help:
                        nc.scalar.copy(num, avs[side][0:HD, :])
                    else:
                        nc.vector.tensor_copy(num, avs[side][0:HD, :])
                    dr = drowp.tile([HD + 1, QC], f32, tag="dr", name="dr")
                    if scalar_help:
                        nc.scalar.copy(dr[HD : HD + 1, :], avs[side][HD : HD + 1, :])
                    else:
                        nc.vector.tensor_copy(
                            dr[HD : HD + 1, :], avs[side][HD : HD + 1, :]
                        )
                    nc.sync.dma_start(
                        out=denom[side : side + 1, :], in_=dr[HD : HD + 1, :]
                    )
                nc.vector.reciprocal_approx_fast(recip, denom)
                nc.vector.tensor_copy(recipb, recip)
                nc.sync.dma_start(out=recip_dram[2 * ui : 2 * ui + 2, :], in_=recipb)
                for side in range(2):
                    h = pair * 2 + side
                    rb = rbp.tile([HD, QC], bf16, tag="rb", name="rb")
                    nc.sync.dma_start(
                        out=rb,
                        in_=recip_dram[2 * ui + side : 2 * ui + side + 1, :].to_broadcast(
                            [HD, QC]
                        ),
                    )
                    if side == 0:
                        nc.vector.tensor_mul(
                            outTP_sb[0:HD, pair, ts(qc, QC)], nums[side], rb
                        )
                    else:
                        tmp = shiftp.tile([HD, QC], bf16, tag="sh", name="sh")
                        nc.vector.tensor_mul(tmp, nums[side], rb)
                        nc.sync.dma_start(
                            out=outTP_sb[HD:128, pair, ts(qc, QC)], in_=tmp
                        )

            fc_state = {}

            def fc_step(qc, scalar_evac=False):
                """One et-chunk of the fc_out partial for q-columns qc*512.."""
                et = fc_state.get(qc, 0)
                if et >= ET:
                    return False
                fc_state[qc] = et + 1
                fps = fc_pool["cur"].tile([128, QC], f32, tag="fc", name=f"fc{qc}_{et}")
                for pair in range(NPAIRS):
                    nc.tensor.matmul(
                        fps,
                        wo_sb[:, pair, et, :],
                        outTP_sb[:, pair, ts(qc, QC)],
                        start=(pair == 0),
                        stop=(pair == NPAIRS - 1),
                    )
                ob = outp.tile([128, QC], bf16, tag="ob", name="ob")
                # ScalarE is exp-free by the time the late chunks evacuate
                if scalar_evac and et % 2 == 0:
                    nc.scalar.copy(ob, fps)
                else:
                    nc.vector.tensor_copy(ob, fps)
                nc.sync.dma_start(out=out_d[ts(et, 128), ts(qc, QC)], in_=ob)
                return True

            # ================= Phase A: projections =================
            nc.vector.memset(v_sb[:, :, :, HD : HD + 1], 1.0)

            # warm the PE clock during the input-DMA wait: dummy matmuls on
            # a memset tile pull the HAM out of the cold p-state before the
            # first real chains
            warm_sb = singles.tile([128, 384], bf16, name="warm_sb")
            nc.vector.memset(warm_sb, 0.0)
            wps = psA.tile([128, QC], f32, tag="ps", name="warm")
            for i in range(16):
                nc.tensor.matmul(
                    wps[:, 0:256], warm_sb[:, 0:128], warm_sb[:, 128:384],
                    start=(i == 0), stop=(i == 15),
                )

            # lc0: j0/j1 chains interleaved per e-tile so each matmul fires
            # as soon as its xT slice lands
            pst01 = [
                psA.tile([128, QC], f32, tag="ps", name=f"qk{j}_0") for j in (0, 1)
            ]
            for et in range(ET):
                for j in (0, 1):
                    nc.tensor.matmul(
                        pst01[j],
                        wqk_sb[:, j, et, :],
                        xT_sb[:, et, 0:QC],
                        start=(et == 0),
                        stop=(et == ET - 1),
                    )
            nc.vector.tensor_copy(qt_sb[:, 0, 0:QC], pst01[0])
            nc.vector.tensor_copy(kt_sb[:, 0, 0:QC], pst01[1])
            qt_done[0].add(0)
            kt_done[0].add(0)

            def proj_chain(j, lc):
                pst = psA.tile([128, QC], f32, tag="ps", name=f"qk{j}_{lc}")
                for et in range(ET):
                    nc.tensor.matmul(
                        pst,
                        wqk_sb[:, j, et, :],
                        xT_sb[:, et, ts(lc, QC)],
                        start=(et == 0),
                        stop=(et == ET - 1),
                    )
                dst = qt_sb if j % 2 == 0 else kt_sb
                nc.vector.tensor_copy(dst[:, j // 2, ts(lc, QC)], pst)
                (qt_done if j % 2 == 0 else kt_done)[j // 2].add(lc)

            try_emit(4)
            for lc in range(1, NLC):
                proj_chain(0, lc)
                proj_chain(1, lc)
                try_emit(4 if lc < 3 else 32)

            # v projections, weaving unit 0's attn@V
            for lt in range(LT):
                pv = psA.tile([128, QC], f32, tag="ps", name=f"v{lt}")
                pv = pv[:, 0 : HPG * HD]
                for et in range(ET):
                    nc.tensor.matmul(
                        pv,
                        xT_sb[:, et, ts(lt, 128)],
                        wv_sb[:, et, :],
                        start=(et == 0),
                        stop=(et == ET - 1),
                    )
                nc.vector.tensor_copy(
                    v_sb[:, lt, :, 0:HD],
                    pv.rearrange("p (h d) -> p h d", h=HPG),
                )
                if lt >= 2:
                    attnv_step(0, lt - 2)
                try_emit(1)

            # j2/j3 (pair-1 q/k projections) interleaved per l-chunk so
            # pair-1 emissions unlock progressively; weave the tail of unit
            # 0 plus unit 1's attn@V
            weave = [(0, 14), (0, 15)] + [(1, k) for k in range(LT)]
            wi = 0

            def weave_step():
                nonlocal wi
                ui, k = weave[wi]
                wi += 1
                attnv_step(ui, k)
                if (ui, k) == (0, 15):
                    finish_unit(0)
                try_emit(1)

            for ci, (lc, j) in enumerate([(lc, j) for lc in range(NLC) for j in (2, 3)]):
                proj_chain(j, lc)
                take = 3 if ci < 2 else 2
                for _ in range(take):
                    if wi < len(weave):
                        weave_step()
            while wi < len(weave):
                weave_step()
            finish_unit(1)
            try_emit(2)

            # phase A projection banks -> fc accumulator banks
            _psA_stack.close()
            _psC_stack = _contextlib.ExitStack()
            fc_pool["cur"] = _psC_stack.enter_context(
                tc.tile_pool(name="psC", bufs=2, space="PSUM")
            )

            # ================= Phase B: remaining units =================
            # Per k-iter: consume exp (ui, k) with 2 attn@V matmuls FIRST
            # (it can never wait on ScalarE - its exp is ~3 units old),
            # weave an fc et-chunk, then emit the next queued scores+exp
            # (which absorbs any wait for the scores PSUM handoff).
            for ui in range(2, 8):
                for k in range(LT):
                    attnv_step(ui, k)
                    if ui >= 5 and k >= 2:
                        fc_step(ui - 5, scalar_evac=(ui >= 6))
                    try_emit(1)
                finish_unit(ui, scalar_help=(ui >= 6))

            assert st["em"] == 128, f"unemitted scores: {st['em']}"
            for qc in range(NQC - 1):  # qc0-2 leftovers (normally none)
                while fc_step(qc, scalar_evac=True):
                    pass

            # tail: last q-chunk's fc on a 6-deep pool (scores + fc banks,
            # both closable barrier-free now).  Pair-0 partials front-load
            # into the accumulators so the PE crunches them while unit 7's
            # normalize chain (3 serial DMA hops) is still producing the
            # pair-1 outTP columns.
            _psC_stack.close()
            _psS_stack.close()
            _psD_stack = _contextlib.ExitStack()
            psD = _psD_stack.enter_context(
                tc.tile_pool(name="psD", bufs=6, space="PSUM")
            )
            accs = [
                psD.tile([128, QC], f32, tag="fcd", name=f"fcd{et}")
                for et in range(6)
            ]
            for et in range(6):
                nc.tensor.matmul(
                    accs[et], wo_sb[:, 0, et, :], outTP_sb[:, 0, ts(3, QC)],
                    start=True, stop=False,
                )

            def fc_tail(et, fps):
                ob = outp.tile([128, QC], bf16, tag="ob", name="ob")
                if et % 2 == 0:
                    nc.scalar.copy(ob, fps)
                else:
                    nc.vector.tensor_copy(ob, fps)
                nc.sync.dma_start(out=out_d[ts(et, 128), ts(3, QC)], in_=ob)

            for et in range(6):
                nc.tensor.matmul(
                    accs[et], wo_sb[:, 1, et, :], outTP_sb[:, 1, ts(3, QC)],
                    start=False, stop=True,
                )
                fc_tail(et, accs[et])
            for et in (6, 7):
                fps = psD.tile([128, QC], f32, tag="fcd", name=f"fcd{et}")
                for pair in range(NPAIRS):
                    nc.tensor.matmul(
                        fps, wo_sb[:, pair, et, :], outTP_sb[:, pair, ts(3, QC)],
                        start=(pair == 0), stop=(pair == NPAIRS - 1),
                    )
                fc_tail(et, fps)
            _psD_stack.close()

            assert len(ex_store) == 0, f"unconsumed exp tiles: {list(ex_store)}"

    nc.compile()
    return nc


def get_nc():
    global _nc_cache
    if _nc_cache is None:
        _nc_cache = build_nc()
    return _nc_cache


def make_core_inputs(x, Wq, Wk, Wv, Wo, bo):
    """Build the 8 per-core input maps from the full-size inputs."""
    x = np.asarray(x, F32)
    Wq = np.asarray(Wq, F32)
    Wk = np.asarray(Wk, F32)
    Wv = np.asarray(Wv, F32)
    Wo = np.asarray(Wo, F32)

    # xT[p, et, l] = x[n].T[et*128 + p, l]
    xT_b = [
        np.ascontiguousarray(x[n].T).reshape(ET, 128, L).transpose(1, 0, 2).astype(BF16)
        for n in range(NB)
    ]

    in_maps = []
    for c in range(NCORES):
        n, g = divmod(c, HPG)
        heads = [g * HPG + i for i in range(HPG)]

        wqk = np.empty((4, EMBED, 128), F32)
        for j in range(4):
            pair, qk = divmod(j, 2)
            hA = heads[2 * pair]
            hB = heads[2 * pair + 1]
            W = Wq if qk == 0 else Wk
            wqk[j, :, 0:HD] = W[hA * HD : (hA + 1) * HD, :].T
            wqk[j, :, HD:128] = W[hB * HD : (hB + 1) * HD, :].T
        wqk8 = wqk.reshape(4, ET, 128, 128).transpose(2, 0, 1, 3).astype(BF16)

        wv = np.concatenate(
            [Wv[h * HD : (h + 1) * HD, :].T for h in heads], axis=1
        )  # [1024, 256]
        wv8 = wv.reshape(ET, 128, HPG * HD).transpose(1, 0, 2).astype(BF16)

        wo = np.empty((NPAIRS, ET, 128, 128), F32)
        for pair in range(NPAIRS):
            hA = heads[2 * pair]
            hB = heads[2 * pair + 1]
            for et in range(ET):
                blk = Wo[et * 128 : (et + 1) * 128, :]
                wo[pair, et, 0:HD, :] = blk[:, hA * HD : (hA + 1) * HD].T
                wo[pair, et, HD:128, :] = blk[:, hB * HD : (hB + 1) * HD].T

        in_maps.append(
            {
                "xT": xT_b[n],
                "wqk": wqk8,
                "wv": wv8,
                "wo": wo.astype(BF16),
            }
        )
    return in_maps


def combine_outputs(results, bo):
    """Sum the per-core fc_out partials, add bias, transpose to [N, L, E]."""
    out = np.empty((NB, L, EMBED), F32)
    for n in range(NB):
        acc = results[n * HPG]["out"].astype(F32)
        for g in range(1, HPG):
            acc = acc + results[n * HPG + g]["out"].astype(F32)
        out[n] = acc.T + np.asarray(bo, F32)
    return out


def kernel(x, Wq, Wk, Wv, Wo, bo):
    global LAST_EXEC_TIME_NS, LAST_RESULTS
    nc = get_nc()
    in_maps = make_core_inputs(x, Wq, Wk, Wv, Wo, bo)
    trace = bool(os.environ.get("KERNEL_TRACE"))
    kw = {}
    if trace:
        kw["trace"] = True
        kw["trace_cores"] = list(range(NCORES))
    res = run_bass_kernel_spmd(nc, in_maps, list(range(NCORES)), **kw)
    LAST_EXEC_TIME_NS = res.exec_time_ns
    LAST_RESULTS = res
    return combine_outputs(res.results, bo)
